# revision 3
# baseline (speedup 1.0000x reference)
"""Trainium2 Bass kernel: hashed-grid embedding lookup.

out[n] = table[h(n)] with
  h = (idx0*1 ^ idx1*19349663 ^ idx2*83492791) mod 2**22

Since NUM_ENTRIES is a power of two, the mod is a bitmask and the hash can
be computed exactly with low-22-bit integer arithmetic.  The DVE ALU does
arithmetic in fp32 (exact only below 2**24), so each prime is split into
two 11-bit chunks; every partial product of an 11-bit coordinate with an
11-bit chunk stays below 2**23 and is exact.  Bitwise ops and shifts are
native integer ops on the DVE.

Sharding: data-parallel across 8 cores; idx/out sharded along the point
dimension, the 64MB table replicated per core.  The gather itself is a
GPSIMD indirect DMA from the DRAM table.

HW note (measured, diverges from CoreSim): the indirect-DMA ucode consumes
ONE offset per contiguous dest run (per partition row), so each
indirect_dma_start gathers exactly 128 rows (one per partition) and costs
~14.4us (dominated by 128 non-pipelined per-partition offset reads on the
Q7).  Multi-run dest APs are garbled by the ucode, and the fast multi-index
dma_gather path requires int16 indices + 256B elements, which this table
cannot satisfy without MoE-style index compaction.  7813 gather
instructions per core => ~113ms HW exec, Pool-engine-bound.
"""

import numpy as np

N_POINTS = 8_000_000
N_DIMS = 3
NUM_ENTRIES = 1 << 22
NUM_FEATURES = 4
N_CORES = 8
P = 128
MASK22 = (1 << 22) - 1

P1 = 19349663
P2 = 83492791
C0 = P1 & 0x7FF          # 159
C1 = (P1 >> 11) & 0x7FF  # 1256
D0 = P2 & 0x7FF          # 1975
D1 = (P2 >> 11) & 0x7FF  # 1855

PTS_PER_CORE = N_POINTS // N_CORES            # 1,000,000
PTS_PAD = ((PTS_PER_CORE + P - 1) // P) * P   # 1,000,064


def build_bass_program(
    npts_pad: int = PTS_PAD,
    K: int = 1024,
    num_devices: int = N_CORES,
    n_entries: int = NUM_ENTRIES,
    repeat: int = 1,
    io_bufs: int = 2,
    tmp_bufs: int = 2,
    fuse_and_shift: bool = True,
):
    """Build + compile the per-core Bass program (SPMD; same on all cores).

    DRAM tensors (per core):
      idx   [128, per_part*6] int32 -- idx int64 bytes viewed as int32 pairs,
                                       point n=(p*per_part+j) at row p, cols 6j..6j+5
      table [n_entries, 4]   f32
      out   [128, per_part*4] f32
    """
    import concourse.bacc as bacc
    import concourse.bass as bass
    import concourse.mybir as mybir
    import concourse.tile as tile

    assert npts_pad % P == 0
    per_part = npts_pad // P
    i32, f32 = mybir.dt.int32, mybir.dt.float32
    Alu = mybir.AluOpType

    nc = bacc.Bacc(
        "TRN2", target_bir_lowering=False, debug=False, num_devices=num_devices
    )
    idx = nc.dram_tensor("idx", [P, per_part * 6], i32, kind="ExternalInput")
    table = nc.dram_tensor(
        "table", [n_entries, NUM_FEATURES], f32, kind="ExternalInput"
    )
    out = nc.dram_tensor("out", [P, per_part * NUM_FEATURES], f32, kind="ExternalOutput")

    # chunk the per-partition range into pieces of K (+ tail)
    chunks = []
    off = 0
    while off < per_part:
        k = min(K, per_part - off)
        chunks.append((off, k))
        off += k

    with tile.TileContext(nc) as tc:
        with (
            tc.tile_pool(name="io", bufs=io_bufs) as io_pool,
            tc.tile_pool(name="tmp", bufs=tmp_bufs) as tmp_pool,
        ):
            for _ in range(repeat):
                for off, k in chunks:
                    idx_t = io_pool.tile([P, 6 * k], i32, tag="idx")
                    out_t = io_pool.tile([P, NUM_FEATURES * k], f32, tag="out")
                    ta = tmp_pool.tile([P, k], i32, tag="ta")
                    tb = tmp_pool.tile([P, k], i32, tag="tb")
                    h = tmp_pool.tile([P, k], i32, tag="h")

                    nc.sync.dma_start(idx_t[:], idx[:, 6 * off : 6 * (off + k)])

                    xv = idx_t[:, 0 :: 6]
                    yv = idx_t[:, 2 :: 6]
                    zv = idx_t[:, 4 :: 6]

                    # A[y] = (y*P1) mod 2^22 (plus <2 high garbage bits)
                    nc.vector.tensor_scalar(tb[:], yv, C1, None, Alu.mult)
                    if fuse_and_shift:
                        nc.vector.tensor_scalar(
                            tb[:], tb[:], 0x7FF, 11, Alu.bitwise_and,
                            Alu.logical_shift_left,
                        )
                    else:
                        nc.vector.tensor_scalar(tb[:], tb[:], 0x7FF, None, Alu.bitwise_and)
                        nc.vector.tensor_scalar(tb[:], tb[:], 11, None, Alu.logical_shift_left)
                    nc.vector.tensor_scalar(ta[:], yv, C0, None, Alu.mult)
                    nc.vector.tensor_tensor(ta[:], ta[:], tb[:], Alu.add)

                    # B[z] = (z*P2) mod 2^22 (plus garbage high bits)
                    nc.vector.tensor_scalar(tb[:], zv, D1, None, Alu.mult)
                    if fuse_and_shift:
                        nc.vector.tensor_scalar(
                            tb[:], tb[:], 0x7FF, 11, Alu.bitwise_and,
                            Alu.logical_shift_left,
                        )
                    else:
                        nc.vector.tensor_scalar(tb[:], tb[:], 0x7FF, None, Alu.bitwise_and)
                        nc.vector.tensor_scalar(tb[:], tb[:], 11, None, Alu.logical_shift_left)
                    nc.vector.tensor_scalar(h[:], zv, D0, None, Alu.mult)
                    nc.vector.tensor_tensor(h[:], h[:], tb[:], Alu.add)

                    # h = (x ^ A ^ B) & MASK22
                    nc.vector.tensor_tensor(h[:], h[:], ta[:], Alu.bitwise_xor)
                    nc.vector.tensor_tensor(h[:], h[:], xv, Alu.bitwise_xor)
                    nc.vector.tensor_scalar(h[:], h[:], MASK22, None, Alu.bitwise_and)

                    # gather rows from the DRAM table: out_t[p, 4j:4j+4] = table[h[p, j]]
                    # HW indirect DMA consumes ONE offset per partition-row
                    # (contiguous dest run), so issue one instruction per
                    # point-column: 128 rows per instruction.
                    for j in range(k):
                        nc.gpsimd.indirect_dma_start(
                            out=out_t[:, NUM_FEATURES * j : NUM_FEATURES * (j + 1)],
                            out_offset=None,
                            in_=table[:],
                            in_offset=bass.IndirectOffsetOnAxis(ap=h[:, j : j + 1], axis=0),
                        )

                    nc.sync.dma_start(
                        out[:, NUM_FEATURES * off : NUM_FEATURES * (off + k)], out_t[:]
                    )

    nc.compile()
    return nc


_CACHE: dict = {}


def _get_program():
    if "nc" not in _CACHE:
        _CACHE["nc"] = build_bass_program()
    return _CACHE["nc"]


def kernel(idx: np.ndarray, table: np.ndarray) -> np.ndarray:
    """idx [8M,3] int64, table [2^22,4] f32 -> out [8M,4] f32."""
    from concourse import bass_utils

    nc = _get_program()

    assert idx.shape == (N_POINTS, N_DIMS) and table.shape == (NUM_ENTRIES, NUM_FEATURES)
    idx = np.ascontiguousarray(idx)
    if idx.dtype == np.int64:
        idx32 = idx.view(np.int32).reshape(N_POINTS, 6)
    else:
        # tolerate an int32 idx: interleave zeros to match the int64 byte layout
        idx32 = np.zeros((N_POINTS, 6), np.int32)
        idx32[:, 0::2] = idx.astype(np.int32)
    table = np.ascontiguousarray(table, dtype=np.float32)

    per_part = PTS_PAD // P
    in_maps = []
    for c in range(N_CORES):
        shard = idx32[c * PTS_PER_CORE : (c + 1) * PTS_PER_CORE]
        pad = np.zeros((PTS_PAD, 6), np.int32)
        pad[:PTS_PER_CORE] = shard
        in_maps.append(
            {"idx": pad.reshape(P, per_part * 6), "table": table}
        )

    res = bass_utils.run_bass_kernel_spmd(
        nc, in_maps, core_ids=list(range(N_CORES))
    )
    outs = []
    for c in range(N_CORES):
        o = res.results[c]["out"].reshape(PTS_PAD, NUM_FEATURES)
        outs.append(o[:PTS_PER_CORE])
    return np.concatenate(outs, axis=0)


def reference_hash(idx: np.ndarray) -> np.ndarray:
    """Host-side hash for verification (int64 exact)."""
    h = idx[:, 0].astype(np.int64)
    h = np.bitwise_xor(h, idx[:, 1].astype(np.int64) * P1) % NUM_ENTRIES
    h = np.bitwise_xor(h, idx[:, 2].astype(np.int64) * P2) % NUM_ENTRIES
    return h
